# revision 1
# baseline (speedup 1.0000x reference)
import sys
for _p in ("/opt/trn_rl_repo",):
    if _p not in sys.path:
        sys.path.insert(0, _p)
"""Bass/Tile multi-head attention kernel for TRN2, head-sharded across 8 cores.

Math (per core c, heads h0=2c, h1=2c+1, dims slice dd = [128c : 128c+128]):
  QT = (Wq[dd] @ q^T + bq[dd])          # [128, R]  (dims on partitions)
  KT, VT likewise; V_aug[bh] = [V_bh | 1]  # [128 k-rows, 65] per (batch, head)
  scoresT = K_h Q_h^T / 8               # [k, q] tiles, dk=64 contraction
  attnT = exp(scoresT)                  # no max-subtraction (scores ~ N(0,1))
  outT_aug = V_aug^T @ attnT            # [65, q]; row 64 = Z (softmax denom)
  concatT_h = outT_aug[:64] / Z         # via matmul-broadcast of Z + recip
  out_projT += Wo[:, dd_h]^T-part @ concatT_h   # [1024, R] partial, host sums

Host: feeds qT/kT/vT (pre-transposed [D, R]), per-core weight slices; sums the
8 partial out_projT results, adds bo, transposes back.

All matmul operands are float32r (TF32-like: ~1.5e-4 mean rel err, full PE rate).
"""
import numpy as np

import concourse.bass as bass
import concourse.bacc as bacc
import concourse.mybir as mybir
import concourse.tile as tile

F32 = mybir.dt.float32
F32R = mybir.dt.float32r
F16 = mybir.dt.float16
EXP = mybir.ActivationFunctionType.Exp

P = 128
DK = 64
D = 1024
DO = D // P          # 8 contraction tiles for projections
NCORES = 8
RBLK = 512           # r-block (free-dim streaming chunk)


def build_kernel(B=2, S=2048, reps=1, in_dt="f32r"):
    """Returns (nc, meta). Same program for all 8 cores (SPMD); per-core data
    differs only in the weight slices fed by the host."""
    R = B * S
    NRB = R // RBLK          # r-blocks total
    RBPB = S // RBLK         # r-blocks per batch
    NKT = S // P             # k-slices (128 rows) per batch
    NCT = D // P             # output column tiles

    DTI = F16 if in_dt == "f16" else F32R
    nc = bacc.Bacc("TRN2", target_bir_lowering=False, debug=False,
                   num_devices=NCORES)

    def din(name, shape, dt=F32R):
        return nc.dram_tensor(name, shape, dt, kind="ExternalInput").ap()

    qT = din("qT", [D, R], DTI)
    kT = din("kT", [D, R], DTI)
    vT = din("vT", [D, R], DTI)
    wqT = din("wqT", [D, P], DTI)
    wkT = din("wkT", [D, P], DTI)
    wvT = din("wvT", [D, P], DTI)
    woT = din("woT", [P, D])
    bq = din("bq", [1, P], DTI)
    bk = din("bk", [1, P], DTI)
    bv = din("bv", [1, P], DTI)
    ones512 = din("ones512", [1, RBLK], DTI)
    onescol = din("onescol", [P, 16])
    onesZ = din("onesZ", [65, DK])
    ident = din("ident", [P, P])
    outT = nc.dram_tensor("outT", [D, R], F32, kind="ExternalOutput").ap()

    qT_r = qT.rearrange("(do p) r -> p do r", p=P)
    kT_r = kT.rearrange("(do p) r -> p do r", p=P)
    vT_r = vT.rearrange("(do p) r -> p do r", p=P)
    outT_r = outT.rearrange("(ct p) r -> p ct r", p=P)

    with tile.TileContext(nc) as tc:
        with tc.tile_pool(name="const", bufs=1) as const, \
             tc.tile_pool(name="io", bufs=3) as io, \
             tc.tile_pool(name="mid", bufs=2) as mid, \
             tc.tile_pool(name="atp", bufs=4) as atp, \
             tc.tile_pool(name="opp", bufs=2) as opp, \
             tc.tile_pool(name="ps_main", bufs=2, space="PSUM") as ps_main, \
             tc.tile_pool(name="ps_sc", bufs=2, space="PSUM") as ps_sc, \
             tc.tile_pool(name="ps_out", bufs=1, space="PSUM") as ps_out:

            # --- constants (early: everything the first q/k projections
            # and first scores need; late: v-transpose / normalize / out-proj
            # constants, emitted after the first input loads) ---
            wq_sb = const.tile([P, DO, P], DTI)
            wk_sb = const.tile([P, DO, P], DTI)
            wv_sb = const.tile([P, DO, P], DTI)
            bq_sb = const.tile([1, P], DTI, tag="bq")
            bk_sb = const.tile([1, P], DTI, tag="bk")
            bv_sb = const.tile([1, P], DTI, tag="bv")
            ones_sb = const.tile([1, RBLK], DTI, tag="ones")
            onesZ_sb = const.tile([65, DK], F32R, tag="onesZ")
            id_sb = const.tile([P, P], F32R, tag="ident")
            wo_sb = [const.tile([DK, NCT, P], F32R, tag=f"wo{h}", name=f"wo{h}") for h in (0, 1)]
            KT_sb = const.tile([P, R], F32R, tag="KT")
            vaug = [const.tile([P, NKT, DK + 1], F32R, tag=f"vaug{bh}", name=f"vaug{bh}")
                    for bh in range(2 * B)]

            nc.sync.dma_start(wq_sb[:], wqT.rearrange("(do p) d -> p do d", p=P))
            nc.sync.dma_start(bq_sb[:], bq)
            nc.sync.dma_start(ones_sb[:], ones512)
            nc.sync.dma_start(wk_sb[:], wkT.rearrange("(do p) d -> p do d", p=P))
            nc.sync.dma_start(bk_sb[:], bk)

            # Warm the PE (HAM clock gate) on already-loaded weight data
            # while the first q/k input DMAs stream; results are discarded.
            def pe_warmup():
                wps = ps_main.tile([P, P], F32, tag="proj", name="warm")
                for i in range(12):
                    nc.tensor.matmul(wps[:], wq_sb[:, i % DO],
                                     wq_sb[:, (i + 1) % DO],
                                     start=True, stop=True)

            def const_late():
                nc.sync.dma_start(wv_sb[:],
                                  wvT.rearrange("(do p) d -> p do d", p=P))
                nc.sync.dma_start(bv_sb[:], bv)
                nc.sync.dma_start(id_sb[:], ident)
                nc.sync.dma_start(onesZ_sb[:], onesZ)
                for h in (0, 1):
                    nc.sync.dma_start(
                        wo_sb[h][:],
                        woT[h * DK:(h + 1) * DK].rearrange("d (ct c) -> d ct c", c=P))
                for bh in range(2 * B):
                    nc.sync.dma_start(vaug[bh][:, :, DK:DK + 1],
                                      onescol[:, 0:NKT, None])

            # out[p_out, free] = lhsT.T @ rhs: lhsT = weight tile (output dims
            # on its free axis), rhs = transposed-activation block (rows on
            # free axis), contraction over D on partitions.
            def proj2(src_r, w_sb, b_sb, rb):
                t = io.tile([P, DO, RBLK], DTI, tag="io")
                nc.sync.dma_start(t[:], src_r[:, :, rb * RBLK:(rb + 1) * RBLK])
                ps = ps_main.tile([P, RBLK], F32, tag="proj")
                for do in range(DO):
                    nc.tensor.matmul(ps[:], w_sb[:, do], t[:, do],
                                     start=(do == 0), stop=False)
                nc.tensor.matmul(ps[:], b_sb[:], ones_sb[:],
                                 start=False, stop=True)
                return ps

            # ---- Stage A: K/V projections + V transpose ----
            # Emitted as closures so stage_b can interleave them between
            # attention steps (keeps PE FIFO from fencing ACT).
            def stage_a_parts(rb):
                b = rb // RBPB
                parts = []

                def kpart():
                    ps_k = proj2(kT_r, wk_sb, bk_sb, rb)
                    nc.vector.tensor_copy(
                        KT_sb[:, rb * RBLK:(rb + 1) * RBLK], ps_k[:])
                parts.append(kpart)

                vt_box = []

                def vpart():
                    ps_v = proj2(vT_r, wv_sb, bv_sb, rb)
                    vt = mid.tile([P, RBLK], F32R, tag="vt", name="vt")
                    nc.vector.tensor_copy(vt[:], ps_v[:])
                    vt_box.append(vt)
                parts.append(vpart)

                def tpart(h, rc):
                    def f():
                        vt = vt_box[0]
                        kt_i = (rb % RBPB) * (RBLK // P) + rc
                        tp = ps_main.tile([P, DK], F32R, tag="proj", name="tp")
                        nc.tensor.transpose(
                            tp[:],
                            vt[h * DK:(h + 1) * DK, rc * P:(rc + 1) * P],
                            id_sb[h * DK:(h + 1) * DK, h * DK:(h + 1) * DK])
                        nc.vector.tensor_copy(
                            vaug[b * 2 + h][:, kt_i, 0:DK], tp[:])
                    return f
                for h in (0, 1):
                    for rc in range(RBLK // P):
                        parts.append(tpart(h, rc))
                return parts

            # ---- Stage B: Q proj + attention + out-projection ----
            qtbs = {}
            pos = {}

            def stage_q_part(rb):
                def f():
                    ps_q = proj2(qT_r, wq_sb, bq_sb, rb)
                    qtb = mid.tile([P, RBLK], F32R, tag="qtb", name="qtb")
                    nc.vector.tensor_copy(qtb[:], ps_q[:])
                    qtbs[rb] = qtb
                return [f]

            def stage_b(rb, prep_pre, prep):
                """Attention ki-loop for rb. `prep_pre` (previous block's
                normalize, which releases its po banks) runs before po
                allocation; `prep` closures interleave between ki steps so PE
                slack under the ACT-bound loop is used for projections and
                out-projections of other blocks."""
                b = rb // RBPB
                qtb = qtbs.pop(rb)
                for p in prep_pre:
                    p()
                po = [ps_out.tile([DK + 1, RBLK], F32, tag=f"po{h}", name=f"po{h}")
                      for h in (0, 1)]
                pos[rb] = po
                pi = 0

                def emit_scexp(ki):
                    # paired heads: one [128, 2, 512] scores tile, one exp
                    scp = ps_sc.tile([P, 2, RBLK], F32, tag="sc", name="sc")
                    for h in (0, 1):
                        hs = slice(h * DK, (h + 1) * DK)
                        nc.tensor.matmul(
                            scp[:, h],
                            KT_sb[hs, b * S + ki * P: b * S + (ki + 1) * P],
                            qtb[hs, :], start=True, stop=True)
                    at = atp.tile([P, 2, RBLK], F32R, tag="at", name="at")
                    nc.scalar.activation(at[:], scp[:], EXP, scale=0.125)
                    return at

                # software-pipelined: sc/exp for ki+1 precede po for ki in
                # the PE FIFO, so ACT never waits on a po-fenced sc.
                pend = emit_scexp(0)
                for ki in range(NKT):
                    if ki + 1 < NKT:
                        nxt = emit_scexp(ki + 1)
                    for h in (0, 1):
                        bh = b * 2 + h
                        nc.tensor.matmul(po[h][:], vaug[bh][:, ki, :],
                                         pend[:, h],
                                         start=(ki == 0), stop=(ki == NKT - 1))
                    if ki + 1 < NKT:
                        pend = nxt
                    # interleave up to 2 prep closures per ki step
                    for _ in range(2):
                        if pi < len(prep):
                            prep[pi]()
                            pi += 1
                for p in prep[pi:]:
                    p()

            def stage_n_parts(rb):
                """Normalize + out-project block rb (runs one cycle later)."""
                cc = []
                parts = []

                osbs = []

                def ncopy(h):
                    def f():
                        po = pos[rb][h]
                        osb = mid.tile([DK + 1, RBLK], F32R, tag=f"osb{h}",
                                       name=f"osb{h}")
                        nc.vector.tensor_copy(osb[:], po[:])
                        osbs.append(osb)
                    return f

                def npart(h):
                    def f():
                        osb = osbs[h]
                        zb = ps_main.tile([DK, RBLK], F32, tag="proj", name="zb")
                        nc.tensor.matmul(zb[:], onesZ_sb[DK:DK + 1, :],
                                         osb[DK:DK + 1, :], start=True, stop=True)
                        zr = mid.tile([DK, RBLK], F32, tag=f"zr{h}", name=f"zr{h}")
                        nc.vector.reciprocal(zr[:], zb[:])
                        c = mid.tile([DK, RBLK], F32R, tag=f"cc{h}", name=f"cc{h}")
                        nc.vector.tensor_mul(c[:], osb[0:DK, :], zr[:])
                        cc.append(c)
                    return f
                parts.append(ncopy(0))
                parts.append(ncopy(1))
                parts.append(npart(0))
                parts.append(npart(1))

                op_box = []

                def alloc_op():
                    op_box.append(opp.tile([P, NCT, RBLK], F32, tag="op_sb",
                                           name="op_sb"))
                parts.append(alloc_op)

                def oppart(ct):
                    def f():
                        op = ps_main.tile([P, RBLK], F32, tag="proj", name="op")
                        nc.tensor.matmul(op[:], wo_sb[0][:, ct], cc[0][:],
                                         start=True, stop=False)
                        nc.tensor.matmul(op[:], wo_sb[1][:, ct], cc[1][:],
                                         start=False, stop=True)
                        nc.vector.tensor_copy(op_box[0][:, ct], op[:])
                    return f
                for ct in range(NCT):
                    parts.append(oppart(ct))

                def store():
                    del pos[rb]
                    nc.gpsimd.dma_start(
                        outT_r[:, :, rb * RBLK:(rb + 1) * RBLK], op_box[0][:])
                parts.append(store)
                return parts

            # Pipeline: cycle i runs attention(i) on ACT/PE while
            # interleaved prep does q-proj(i+1/i+2), batch-1 K/V proj, and
            # the deferred normalize+out-proj of block i-1.
            for rep in range(reps):
                for f in stage_q_part(0):
                    f()
                if rep == 0:
                    pe_warmup()
                a0 = stage_a_parts(0)
                a0[0]()          # k-projection of block 0 right away
                if rep == 0:
                    const_late()
                for f in a0[1:]:
                    f()
                for rb in range(1, RBPB):
                    for f in stage_a_parts(rb):
                        f()
                for f in stage_q_part(1):
                    f()
                # Batch-1 K/V prep: one block per cycle starting at rb1 —
                # rb0's PE FIFO must stay clear of matmuls whose input DMA
                # queues behind the batch-0 loads.
                amap = {rb: [RBPB + rb - 1] for rb in range(1, RBPB + 2)}
                for rb in range(NRB):
                    prep_pre = []
                    prep = []
                    if rb >= 1:
                        nparts = stage_n_parts(rb - 1)
                        prep_pre = nparts[:2]     # po -> SBUF copies (release)
                        prep += nparts[2:]        # normalize + out-proj + store
                    if rb + 2 < NRB:
                        prep += stage_q_part(rb + 2)
                    for a_rb in amap.get(rb, []):
                        if RBPB <= a_rb < NRB:
                            prep += stage_a_parts(a_rb)
                    stage_b(rb, prep_pre, prep)
                for f in stage_n_parts(NRB - 1):
                    f()

    nc.compile()
    return nc


def host_prepare(q, k, v, Wq, bq, Wk, bk, Wv, bv, Wo, bo, B=2, S=2048,
                 in_dt="f32r"):
    """Build per-core in_maps. Returns (in_maps, postprocess)."""
    R = B * S
    f32 = np.float32
    fin = np.float16 if in_dt == "f16" else f32
    qT = np.ascontiguousarray(q.reshape(R, D).T).astype(fin, copy=False)
    kT = np.ascontiguousarray(k.reshape(R, D).T).astype(fin, copy=False)
    vT = np.ascontiguousarray(v.reshape(R, D).T).astype(fin, copy=False)
    ones512 = np.ones((1, RBLK), fin)
    onesZ = np.ones((65, DK), f32)
    ident = np.eye(P, dtype=f32)
    Wq, Wk, Wv, Wo = (np.asarray(x, f32) for x in (Wq, Wk, Wv, Wo))
    bqa, bka, bva, boa = (np.asarray(x, f32) for x in (bq, bk, bv, bo))

    in_maps = []
    for c in range(NCORES):
        dd = slice(P * c, P * (c + 1))
        in_maps.append({
            "qT": qT, "kT": kT, "vT": vT,
            "wqT": np.ascontiguousarray(Wq[dd].T).astype(fin, copy=False),
            "wkT": np.ascontiguousarray(Wk[dd].T).astype(fin, copy=False),
            "wvT": np.ascontiguousarray(Wv[dd].T).astype(fin, copy=False),
            "woT": np.ascontiguousarray(Wo[:, dd].T),
            "bq": bqa[dd].reshape(1, P).astype(fin, copy=False),
            "bk": bka[dd].reshape(1, P).astype(fin, copy=False),
            "bv": bva[dd].reshape(1, P).astype(fin, copy=False),
            "ones512": ones512, "onesZ": onesZ, "ident": ident,
            "onescol": np.ones((P, 16), f32),
        })

    def postprocess(results):
        acc = np.zeros((D, R), np.float64)
        for c in range(NCORES):
            acc += results[c]["outT"]
        out = acc.T + boa.astype(np.float64)
        return out.astype(f32).reshape(B, S, D)

    return in_maps, postprocess


# ---------------------------------------------------------------------------
# Harness entry point: full inputs in, full output out.
# ---------------------------------------------------------------------------
_BUILD_CACHE = {}


def kernel(q, k, v, Wq, bq, Wk, bk, Wv, bv, Wo, bo, mask=0, **_unused):
    from concourse import bass_utils

    nc = _BUILD_CACHE.get("nc")
    if nc is None:
        nc = build_kernel(B=2, S=2048)
        _BUILD_CACHE["nc"] = nc

    args = [np.asarray(x, np.float32) for x in
            (q, k, v, Wq, bq, Wk, bk, Wv, bv, Wo, bo)]
    in_maps, post = host_prepare(*args)
    res = bass_utils.run_bass_kernel_spmd(nc, in_maps, core_ids=list(range(8)))
    return post(res.results)



# revision 13
# speedup vs baseline: 3.9783x; 3.9783x over previous
import sys
for _p in ("/opt/trn_rl_repo",):
    if _p not in sys.path:
        sys.path.insert(0, _p)
"""Bass/Tile multi-head attention kernel for TRN2, head-sharded across 8 cores.

Math (per core c, heads h0=2c, h1=2c+1, dims slice dd = [128c : 128c+128]):
  QT = (Wq[dd] @ q^T + bq[dd])          # [128, R]  (dims on partitions)
  KT, VT likewise; V_aug[bh] = [V_bh | 1]  # [128 k-rows, 65] per (batch, head)
  scoresT = K_h Q_h^T / 8               # [k, q] tiles, dk=64 contraction
  attnT = exp(scoresT)                  # no max-subtraction (scores ~ N(0,1))
  outT_aug = V_aug^T @ attnT            # [65, q]; row 64 = Z (softmax denom)
  concatT[h*64:(h+1)*64] = outT_aug[:64] / Z   # stacked both heads [128, q]
  out_projT = Wo[:, dd]^T-part @ concatT       # [1024, R] partial, host sums

Host: feeds qT/kT/vT (pre-transposed [D, R], f16), per-core weight slices
(f16); sums the 8 f16 partial out_projT results, adds bo, transposes back.

f16 operands run the PE at full rate (1 col/cycle); intermediate tensors are
f16 to halve DMA + SBUF traffic (rel err ~1e-3, budget 2e-2).
"""
import numpy as np

import concourse.bass as bass
import concourse.bacc as bacc
import concourse.mybir as mybir
import concourse.tile as tile

F32 = mybir.dt.float32
F32R = mybir.dt.float32r
F16 = mybir.dt.float16
EXP = mybir.ActivationFunctionType.Exp

P = 128
DK = 64
D = 1024
DO = D // P          # 8 contraction tiles for projections
NCORES = 8
RBLK = 512           # r-block (free-dim streaming chunk)


def build_kernel(B=2, S=2048, reps=1, in_dt="f16"):
    """Returns (nc, meta). Same program for all 8 cores (SPMD); per-core data
    differs only in the weight slices fed by the host."""
    R = B * S
    NRB = R // RBLK          # r-blocks total
    RBPB = S // RBLK         # r-blocks per batch
    NKT = S // P             # k-slices (128 rows) per batch
    NCT = D // P             # output column tiles

    DTI = F16 if in_dt == "f16" else F32R
    DTM = F16                # intermediate dtype (SBUF tensors fed to PE)
    nc = bacc.Bacc("TRN2", target_bir_lowering=False, debug=False,
                   num_devices=NCORES)

    def din(name, shape, dt=DTM):
        return nc.dram_tensor(name, shape, dt, kind="ExternalInput").ap()

    qT = din("qT", [D, R], DTI)
    kT = din("kT", [D, R], DTI)
    vT = din("vT", [D, R], DTI)
    # weight slices pre-packed host-side as [p, do*P] so each DMA row is one
    # contiguous 2KB descriptor (256B descriptors pay a 2x DMA latency)
    wqT = din("wqT", [P, DO * P], DTI)
    wkT = din("wkT", [P, DO * P], DTI)
    wvT = din("wvT", [P, DO * P], DTI)
    woT = din("woT", [P, D], DTM)
    bq = din("bq", [1, P], DTI)
    bk = din("bk", [1, P], DTI)
    bv = din("bv", [1, P], DTI)
    ones512 = din("ones512", [1, RBLK], DTI)
    onesZ = din("onesZ", [65, DK], DTM)
    ident = din("ident", [P, P], DTM)
    outT = nc.dram_tensor("outT", [D, R], F16, kind="ExternalOutput").ap()

    qT_r = qT.rearrange("(do p) r -> p do r", p=P)
    kT_r = kT.rearrange("(do p) r -> p do r", p=P)
    vT_r = vT.rearrange("(do p) r -> p do r", p=P)
    outT_r = outT.rearrange("(ct p) r -> p ct r", p=P)

    with tile.TileContext(nc) as tc:
        with tc.tile_pool(name="const", bufs=1) as const, \
             tc.tile_pool(name="io", bufs=5) as io, \
             tc.tile_pool(name="mid", bufs=2) as mid, \
             tc.tile_pool(name="atp", bufs=4) as atp, \
             tc.tile_pool(name="opp", bufs=2) as opp, \
             tc.tile_pool(name="ps_main", bufs=2, space="PSUM") as ps_main, \
             tc.tile_pool(name="ps_sc", bufs=2, space="PSUM") as ps_sc, \
             tc.tile_pool(name="ps_out", bufs=1, space="PSUM") as ps_out:

            # --- constants (early: everything the first q/k projections
            # and first scores need; late: v-transpose / normalize / out-proj
            # constants, emitted after the first input loads) ---
            wq_sb = const.tile([P, DO, P], DTI)
            wk_sb = const.tile([P, DO, P], DTI)
            wv_sb = const.tile([P, DO, P], DTI)
            bq_sb = const.tile([1, P], DTI, tag="bq")
            bk_sb = const.tile([1, P], DTI, tag="bk")
            bv_sb = const.tile([1, P], DTI, tag="bv")
            ones_sb = const.tile([1, RBLK], DTI, tag="ones")
            onesZ_sb = const.tile([65, DK], DTM, tag="onesZ")
            id_sb = const.tile([P, P], DTM, tag="ident")
            wo_sb = const.tile([P, NCT, P], DTM, tag="wo", name="wo")
            KT_sb = const.tile([P, R], DTM, tag="KT")
            vaug = [const.tile([P, NKT, DK + 1], DTM, tag=f"vaug{bh}", name=f"vaug{bh}")
                    for bh in range(2 * B)]

            nc.sync.dma_start(wq_sb[:], wqT.rearrange("p (do d) -> p do d", do=DO))
            nc.sync.dma_start(bq_sb[:], bq)
            nc.sync.dma_start(ones_sb[:], ones512)
            nc.sync.dma_start(id_sb[:], ident)
            nc.sync.dma_start(onesZ_sb[:], onesZ)
            nc.sync.dma_start(wk_sb[:], wkT.rearrange("p (do d) -> p do d", do=DO))
            nc.sync.dma_start(bk_sb[:], bk)

            # Warm the PE (HAM clock gate) on already-loaded weight data
            # while the first q/k input DMAs stream; results are discarded.
            def pe_warmup():
                wps = ps_main.tile([P, P], F32, tag="proj", name="warm")
                for i in range(12):
                    nc.tensor.matmul(wps[:], wq_sb[:, i % DO],
                                     wq_sb[:, (i + 1) % DO],
                                     start=True, stop=True)

            def const_late():
                nc.sync.dma_start(wv_sb[:],
                                  wvT.rearrange("p (do d) -> p do d", do=DO))
                nc.sync.dma_start(bv_sb[:], bv)
                nc.sync.dma_start(
                    wo_sb[:], woT.rearrange("d (ct c) -> d ct c", c=P))
                for bh in range(2 * B):
                    nc.gpsimd.memset(vaug[bh][:, :, DK:DK + 1], 1.0)

            # out[p_out, free] = lhsT.T @ rhs: lhsT = weight tile (output dims
            # on its free axis), rhs = transposed-activation block (rows on
            # free axis), contraction over D on partitions.
            def proj2(src_r, w_sb, b_sb, rb):
                t = io.tile([P, DO, RBLK], DTI, tag="io")
                nc.sync.dma_start(t[:], src_r[:, :, rb * RBLK:(rb + 1) * RBLK])
                ps = ps_main.tile([P, RBLK], F32, tag="proj")
                for do in range(DO):
                    nc.tensor.matmul(ps[:], w_sb[:, do], t[:, do],
                                     start=(do == 0), stop=False)
                nc.tensor.matmul(ps[:], b_sb[:], ones_sb[:],
                                 start=False, stop=True)
                return ps

            # ---- Stage A: K/V projections + V transpose ----
            # Emitted as closures so stage_b can interleave them between
            # attention steps (keeps PE FIFO from fencing ACT).
            def stage_a_parts(rb):
                b = rb // RBPB
                parts = []

                def kpart():
                    ps_k = proj2(kT_r, wk_sb, bk_sb, rb)
                    nc.vector.tensor_copy(
                        KT_sb[:, rb * RBLK:(rb + 1) * RBLK], ps_k[:])
                parts.append(kpart)

                vt_box = []

                def vpart():
                    ps_v = proj2(vT_r, wv_sb, bv_sb, rb)
                    vt = mid.tile([P, RBLK], DTM, tag="vt", name="vt")
                    nc.vector.tensor_copy(vt[:], ps_v[:])
                    vt_box.append(vt)
                parts.append(vpart)

                def tpart(h, rc):
                    def f():
                        vt = vt_box[0]
                        kt_i = (rb % RBPB) * (RBLK // P) + rc
                        tp = ps_main.tile([P, DK], DTM, tag="proj", name="tp")
                        nc.tensor.transpose(
                            tp[:],
                            vt[h * DK:(h + 1) * DK, rc * P:(rc + 1) * P],
                            id_sb[h * DK:(h + 1) * DK, h * DK:(h + 1) * DK])
                        nc.vector.tensor_copy(
                            vaug[b * 2 + h][:, kt_i, 0:DK], tp[:])
                    return f
                for h in (0, 1):
                    for rc in range(RBLK // P):
                        parts.append(tpart(h, rc))
                return parts

            # ---- Stage B: Q proj + attention + out-projection ----
            qtbs = {}
            pos = {}

            def stage_q_part(rb):
                def f():
                    ps_q = proj2(qT_r, wq_sb, bq_sb, rb)
                    qtb = mid.tile([P, RBLK], DTM, tag="qtb", name="qtb")
                    nc.vector.tensor_copy(qtb[:], ps_q[:])
                    qtbs[rb] = qtb
                return [f]

            def stage_b(rb, prep_pre, prep):
                """Attention ki-loop for rb. `prep_pre` (previous block's
                normalize, which releases its po banks) runs before po
                allocation; `prep` closures interleave between ki steps so PE
                slack under the ACT-bound loop is used for projections and
                out-projections of other blocks."""
                b = rb // RBPB
                qtb = qtbs.pop(rb)
                pi = 0

                def emit_scexp(ki):
                    # paired heads: one [128, 2, 512] scores tile, one exp
                    scp = ps_sc.tile([P, 2, RBLK], F32, tag="sc", name="sc")
                    for h in (0, 1):
                        hs = slice(h * DK, (h + 1) * DK)
                        nc.tensor.matmul(
                            scp[:, h],
                            KT_sb[hs, b * S + ki * P: b * S + (ki + 1) * P],
                            qtb[hs, :], start=True, stop=True)
                    at = atp.tile([P, 2, RBLK], DTM, tag="at", name="at")
                    nc.scalar.activation(at[:], scp[:], EXP, scale=0.125)
                    return at

                # software-pipelined 2 deep: sc/exp for ki+1 and ki+2 precede
                # po-dependent attnV matmuls in the PE FIFO, so the PE keeps
                # streaming while the previous block's po banks drain and ACT
                # never waits on a po-fenced sc.
                pend = emit_scexp(0)
                nxt = emit_scexp(1) if NKT > 1 else None
                for p in prep_pre:
                    p()
                po = [ps_out.tile([DK + 1, RBLK], F32, tag=f"po{h}", name=f"po{h}")
                      for h in (0, 1)]
                pos[rb] = po
                for ki in range(NKT):
                    cur = pend
                    pend = nxt
                    if ki + 2 < NKT:
                        nxt = emit_scexp(ki + 2)
                    for h in (0, 1):
                        bh = b * 2 + h
                        nc.tensor.matmul(po[h][:], vaug[bh][:, ki, :],
                                         cur[:, h],
                                         start=(ki == 0), stop=(ki == NKT - 1))
                    # interleave up to 2 prep closures per ki step
                    for _ in range(2):
                        if pi < len(prep):
                            prep[pi]()
                            pi += 1
                for p in prep[pi:]:
                    p()

            def stage_n_parts(rb):
                """Normalize + out-project block rb (runs one cycle later)."""
                parts = []

                osbs = []

                def ncopy(h):
                    def f():
                        po = pos[rb][h]
                        osb = mid.tile([DK + 1, RBLK], DTM, tag=f"osb{h}",
                                       name=f"osb{h}")
                        nc.vector.tensor_copy(osb[:], po[:])
                        osbs.append(osb)
                    return f

                cc_box = []

                def npart(h):
                    def f():
                        if not cc_box:
                            cc_box.append(mid.tile([P, RBLK], DTM, tag="cc",
                                                   name="cc"))
                        osb = osbs[h]
                        zb = ps_main.tile([DK, RBLK], F32, tag="proj", name="zb")
                        nc.tensor.matmul(zb[:], onesZ_sb[DK:DK + 1, :],
                                         osb[DK:DK + 1, :], start=True, stop=True)
                        zr = mid.tile([DK, RBLK], DTM, tag=f"zr{h}", name=f"zr{h}")
                        with nc.allow_low_precision(reason="1/Z in f16: rel 5e-4, budget 2e-2"):
                            nc.vector.reciprocal(zr[:], zb[:])
                        # stacked concat: head h lands on partitions
                        # [64h, 64h+64) so the out-proj contracts both heads
                        # in a single 128-deep matmul per column tile.
                        nc.vector.tensor_mul(
                            cc_box[0][h * DK:(h + 1) * DK, :],
                            osb[0:DK, :], zr[:])
                    return f
                parts.append(ncopy(0))
                parts.append(ncopy(1))
                parts.append(npart(0))
                parts.append(npart(1))

                op_box = []

                def alloc_op():
                    op_box.append(opp.tile([P, NCT, RBLK], F16, tag="op_sb",
                                           name="op_sb"))
                parts.append(alloc_op)

                def oppart(ct):
                    def f():
                        op = ps_main.tile([P, RBLK], F32, tag="proj", name="op")
                        nc.tensor.matmul(op[:], wo_sb[:, ct], cc_box[0][:],
                                         start=True, stop=True)
                        nc.vector.tensor_copy(op_box[0][:, ct], op[:])
                    return f
                for ct in range(NCT):
                    parts.append(oppart(ct))

                def store_half(lo, hi):
                    def f():
                        if hi == NCT:
                            del pos[rb]
                        nc.gpsimd.dma_start(
                            outT_r[:, lo:hi, rb * RBLK:(rb + 1) * RBLK],
                            op_box[0][:, lo:hi])
                    return f
                # split the store so the first half streams while the
                # second half's out-proj matmuls still run (shrinks the
                # final-block tail and smooths DMA write bursts)
                parts.insert(5 + NCT // 2, store_half(0, NCT // 2))
                parts.append(store_half(NCT // 2, NCT))
                return parts

            # Pipeline: cycle i runs attention(i) on ACT/PE while
            # interleaved prep does q-proj(i+1/i+2), batch-1 K/V proj, and
            # the deferred normalize+out-proj of block i-1.
            for rep in range(reps):
                if rep == 0:
                    pe_warmup()   # before q-proj: runs off wq alone while q0 loads
                for f in stage_q_part(0):
                    f()
                a0 = stage_a_parts(0)
                a0[0]()          # k-projection of block 0 right away
                if rep == 0:
                    const_late()
                for f in a0[1:]:
                    f()
                for rb in range(1, RBPB):
                    for f in stage_a_parts(rb):
                        f()
                for f in stage_q_part(1):
                    f()
                # Batch-1 K/V prep: one block per cycle starting at rb1 —
                # rb0's PE FIFO must stay clear of matmuls whose input DMA
                # queues behind the batch-0 loads.
                amap = {rb: [RBPB + rb - 1] for rb in range(1, RBPB + 2)}
                for rb in range(NRB):
                    prep_pre = []
                    prep = []
                    if rb >= 1:
                        nparts = stage_n_parts(rb - 1)
                        prep_pre = nparts[:2]     # po -> SBUF copies (release)
                        prep += nparts[2:]        # normalize + out-proj + store
                    if rb + 2 < NRB:
                        prep += stage_q_part(rb + 2)
                    for a_rb in amap.get(rb, []):
                        if RBPB <= a_rb < NRB:
                            prep += stage_a_parts(a_rb)
                    stage_b(rb, prep_pre, prep)
                for f in stage_n_parts(NRB - 1):
                    f()

    nc.compile()
    return nc


def host_prepare(q, k, v, Wq, bq, Wk, bk, Wv, bv, Wo, bo, B=2, S=2048,
                 in_dt="f16"):
    """Build per-core in_maps. Returns (in_maps, postprocess)."""
    R = B * S
    f32 = np.float32
    f16 = np.float16
    fin = f16 if in_dt == "f16" else f32
    qT = np.ascontiguousarray(q.reshape(R, D).T).astype(fin, copy=False)
    kT = np.ascontiguousarray(k.reshape(R, D).T).astype(fin, copy=False)
    vT = np.ascontiguousarray(v.reshape(R, D).T).astype(fin, copy=False)
    ones512 = np.ones((1, RBLK), fin)
    onesZ = np.ones((65, DK), f16)
    ident = np.eye(P, dtype=f16)
    Wq, Wk, Wv, Wo = (np.asarray(x, f32) for x in (Wq, Wk, Wv, Wo))
    bqa, bka, bva, boa = (np.asarray(x, f32) for x in (bq, bk, bv, bo))

    def pack_w(w_slice):
        # W[dd] is [128 outdims, D]; kernel wants tile [p, do, d] with
        # wT[do*128+p, d] = W[d_out=d?]... : wT = W[dd].T -> [D, 128];
        # tile[p, do, d] = wT[do*128+p, d]; pack rows p-major for 2KB DMAs.
        wT = w_slice.T.reshape(DO, P, P)          # [do, p, d]
        return np.ascontiguousarray(
            wT.transpose(1, 0, 2).reshape(P, DO * P)).astype(fin, copy=False)

    in_maps = []
    for c in range(NCORES):
        dd = slice(P * c, P * (c + 1))
        in_maps.append({
            "qT": qT, "kT": kT, "vT": vT,
            "wqT": pack_w(Wq[dd]),
            "wkT": pack_w(Wk[dd]),
            "wvT": pack_w(Wv[dd]),
            "woT": np.ascontiguousarray(Wo[:, dd].T).astype(f16, copy=False),
            "bq": bqa[dd].reshape(1, P).astype(fin, copy=False),
            "bk": bka[dd].reshape(1, P).astype(fin, copy=False),
            "bv": bva[dd].reshape(1, P).astype(fin, copy=False),
            "ones512": ones512, "onesZ": onesZ, "ident": ident,
        })

    def postprocess(results):
        acc = np.zeros((D, R), np.float32)
        for c in range(NCORES):
            acc += results[c]["outT"].astype(np.float32)
        out = acc.T + boa
        return out.astype(f32).reshape(B, S, D)

    return in_maps, postprocess


# ---------------------------------------------------------------------------
# Harness entry point: full inputs in, full output out.
# ---------------------------------------------------------------------------
_BUILD_CACHE = {}


def kernel(q, k, v, Wq, bq, Wk, bk, Wv, bv, Wo, bo, mask=0, **_unused):
    from concourse import bass_utils

    nc = _BUILD_CACHE.get("nc")
    if nc is None:
        nc = build_kernel(B=2, S=2048)
        _BUILD_CACHE["nc"] = nc

    args = [np.asarray(x, np.float32) for x in
            (q, k, v, Wq, bq, Wk, bk, Wv, bv, Wo, bo)]
    in_maps, post = host_prepare(*args)
    res = bass_utils.run_bass_kernel_spmd(nc, in_maps, core_ids=list(range(8)))
    return post(res.results)
